# revision 10
# baseline (speedup 1.0000x reference)
"""Trainium2 Bass kernel for nn_CrossPath_V2 (dense_transformer).

Sharding: 8 NeuronCores, core = 2*b + p  (b in 0..3 batches, p in {0,1} paths).
Each core independently computes out_p[b] = layernorm(x_p[b] + [s_p[b], a_p[b]] @ We_p + be_p).
No cross-core communication: the tiny cross-path coupling (ctx matrices) is handled
by each core computing BOTH channel projections for its batch (c_own for q, c_oth
for k/v), a ~19% FLOP overhead that buys an embarrassingly-parallel kernel.

Device layout strategy ("transposed world"):
  - activations feeding matmul's moving side are pre-transposed on HOST (free numpy)
  - c-proj produces ctT/qT [feat, tok] directly (weights stationary)
  - kv proj flips back to [tok, feat] using ctT tiles as the stationary operand
  - G_h = k_h^T v_h accumulated over 32 token blocks in one PSUM bank, with the
    8 heads packed as [d-pair partitions, e-pair columns] using col tile_position
  - softmax over d folded into: exp on ACT (scale=D^-0.5), column sums via a
    ones-matmul, and a per-partition reciprocal multiply fused into the PSUM evict
  - exp(G) stored block-diagonally per head-pair so q@ctx runs as full
    128x128x512 matmuls (4 per token chunk instead of 8 quarter-utilization ones)
  - end-proj accumulates 8 K-chunks (4 from sT, 4 from aT) into [tok, feat] PSUM,
    then layernorm runs along the free axis (E[x^2]-mu^2 formulation)
"""

import os
import numpy as np
import ml_dtypes

B, N, C, H = 4, 4096, 512, 8
D = C // H
EPS = 1e-5
NCORES = 8
P = 128
CHUNK = 512          # moving-dim token chunk
NCH = N // CHUNK     # 8
FI = C // P          # 4 feature chunks of 128
NTB = CHUNK // P     # 4 token blocks per chunk
NBLK = N // P        # 32 token blocks total

BF16 = ml_dtypes.bfloat16

_CACHE = {}
LAST_RESULTS = None  # BassKernelResults of the most recent run (for test.py)


def _build_nc():
    import concourse.bass as bass
    import concourse.mybir as mybir
    import concourse.tile as tile
    from concourse import bacc
    from contextlib import ExitStack

    dt = mybir.dt
    f32, bf16 = dt.float32, dt.bfloat16
    AF = mybir.ActivationFunctionType
    OP = mybir.AluOpType

    nc = bacc.Bacc("TRN2", target_bir_lowering=False, debug=False,
                   num_devices=NCORES)

    # ---- DRAM parameters (per-core SPMD inputs) ----
    cT_own = nc.dram_tensor("cT_own", [C, N], bf16, kind="ExternalInput").ap()
    cT_oth = nc.dram_tensor("cT_oth", [C, N], bf16, kind="ExternalInput").ap()
    sT = nc.dram_tensor("sT", [C, N], bf16, kind="ExternalInput").ap()
    x = nc.dram_tensor("x", [N, C], f32, kind="ExternalInput").ap()
    W_own = nc.dram_tensor("W_own", [C, C], bf16, kind="ExternalInput").ap()
    W_oth = nc.dram_tensor("W_oth", [C, C], bf16, kind="ExternalInput").ap()
    b_own = nc.dram_tensor("b_own", [P, FI], f32, kind="ExternalInput").ap()
    b_oth = nc.dram_tensor("b_oth", [P, FI], f32, kind="ExternalInput").ap()
    Wkv = nc.dram_tensor("Wkv", [C, 2 * C], bf16, kind="ExternalInput").ap()
    We = nc.dram_tensor("We", [2 * C, C], bf16, kind="ExternalInput").ap()
    g_rep = nc.dram_tensor("g_rep", [P, C], f32, kind="ExternalInput").ap()
    bt_rep = nc.dram_tensor("bt_rep", [P, C], f32, kind="ExternalInput").ap()
    out = nc.dram_tensor("out", [N, C], f32, kind="ExternalOutput").ap()

    with tile.TileContext(nc) as tc, ExitStack() as ctx:
        wpool = ctx.enter_context(tc.tile_pool(name="weights", bufs=1))
        cpool = ctx.enter_context(tc.tile_pool(name="cin", bufs=2))
        ctq = ctx.enter_context(tc.tile_pool(name="ctq", bufs=2))
        kvpool = ctx.enter_context(tc.tile_pool(name="kvsb", bufs=3))
        spool = ctx.enter_context(tc.tile_pool(name="sin", bufs=2))
        xpool = ctx.enter_context(tc.tile_pool(name="xin", bufs=2))
        apool = ctx.enter_context(tc.tile_pool(name="asb", bufs=2))
        opool = ctx.enter_context(tc.tile_pool(name="oln", bufs=3))
        ps_proj = ctx.enter_context(tc.tile_pool(name="psproj", bufs=2, space="PSUM"))
        ps_work = ctx.enter_context(tc.tile_pool(name="pswork", bufs=3, space="PSUM"))
        ps_G = ctx.enter_context(tc.tile_pool(name="psG", bufs=1, space="PSUM"))

        # ---- resident weights ----
        Wown_t, Woth_t, Wkv_t, We_t = [], [], [], []
        for i in range(FI):
            t = wpool.tile([P, C], bf16, tag=f"wown{i}")
            nc.sync.dma_start(t[:], W_own[i * P:(i + 1) * P, :])
            Wown_t.append(t)
            t = wpool.tile([P, C], bf16, tag=f"woth{i}")
            nc.sync.dma_start(t[:], W_oth[i * P:(i + 1) * P, :])
            Woth_t.append(t)
            t = wpool.tile([P, 2 * C], bf16, tag=f"wkv{i}")
            nc.sync.dma_start(t[:], Wkv[i * P:(i + 1) * P, :])
            Wkv_t.append(t)
        for j in range(2 * FI):
            t = wpool.tile([P, C], bf16, tag=f"we{j}")
            nc.sync.dma_start(t[:], We[j * P:(j + 1) * P, :])
            We_t.append(t)
        bown_t = wpool.tile([P, FI], f32, tag="bown")
        nc.sync.dma_start(bown_t[:], b_own[:, :])
        both_t = wpool.tile([P, FI], f32, tag="both")
        nc.sync.dma_start(both_t[:], b_oth[:, :])
        g_t = wpool.tile([P, C], f32, tag="g")
        nc.sync.dma_start(g_t[:], g_rep[:, :])
        bt_t = wpool.tile([P, C], f32, tag="bt")
        nc.sync.dma_start(bt_t[:], bt_rep[:, :])
        ones_t = wpool.tile([P, 1], bf16, tag="ones")
        nc.vector.memset(ones_t[:], 1.0)
        eps_t = wpool.tile([P, 1], f32, tag="eps")
        nc.vector.memset(eps_t[:], EPS)
        expg = []
        for j in range(FI):
            t = wpool.tile([P, P], bf16, tag=f"eg{j}")
            nc.vector.memset(t[:], 0.0)
            expg.append(t)

        # G accumulators: even heads in partitions 0-63 of bank A, odd heads in
        # partitions 64-127 of bank B (separate banks keep one PSUM accumulation
        # group per zero region); cols (h//2)*64..+64
        G_even = ps_G.tile([P, D * FI], f32, tag="Ge")
        G_odd = ps_G.tile([P, D * FI], f32, tag="Go")

        def g_ap(h):
            j, r = h // 2, (h % 2) * D
            t = G_even if h % 2 == 0 else G_odd
            return t[r:r + D, j * D:(j + 1) * D], r

        # ================= PASS 1: c_oth -> ct -> kv -> G =================
        for ch in range(NCH):
            cin = []
            for i in range(FI):
                t = cpool.tile([P, CHUNK], bf16, tag=f"cin{i}")
                nc.sync.dma_start(t[:], cT_oth[i * P:(i + 1) * P,
                                               ch * CHUNK:(ch + 1) * CHUNK])
                cin.append(t)
            ctt = []
            for fo in range(FI):
                pst = ps_proj.tile([P, CHUNK], f32, tag="pp")
                for fi in range(FI):
                    nc.tensor.matmul(pst[:], Woth_t[fi][:, fo * P:(fo + 1) * P],
                                     cin[fi][:], start=(fi == 0), stop=(fi == FI - 1))
                t = ctq.tile([P, CHUNK], bf16, tag=f"ct{fo}")
                nc.scalar.activation(t[:], pst[:], AF.Relu,
                                     bias=both_t[:, fo:fo + 1])
                ctt.append(t)
            for tb in range(NTB):
                blk = ch * NTB + tb
                ksb = kvpool.tile([P, C], bf16, tag="k")
                vsb = kvpool.tile([P, C], bf16, tag="v")
                for half, dst in ((0, ksb), (1, vsb)):
                    pkv = ps_work.tile([P, CHUNK], f32, tag="pw")
                    for fi in range(FI):
                        nc.tensor.matmul(pkv[:], ctt[fi][:, tb * P:(tb + 1) * P],
                                         Wkv_t[fi][:, half * C:(half + 1) * C],
                                         start=(fi == 0), stop=(fi == FI - 1))
                    nc.vector.tensor_copy(dst[:], pkv[:])
                for h in range(H):
                    gout, r = g_ap(h)
                    nc.tensor.matmul(gout,
                                     ksb[:, h * D:(h + 1) * D],
                                     vsb[:, h * D:(h + 1) * D],
                                     start=(blk == 0 and h < 2),
                                     stop=(blk == NBLK - 1 and h >= H - 2),
                                     tile_position=(0, r))

        # ---- softmax pieces: expg (block-diag per pair), column sums, recip ----
        for h in range(H):
            j, r = h // 2, (h % 2) * D
            gin, _ = g_ap(h)
            nc.scalar.activation(expg[j][r:r + D, r:r + D], gin,
                                 AF.Exp, scale=float(D ** -0.5))
        s_ps = ps_G.tile([P, FI], f32, tag="scol")
        for j in range(FI):
            nc.tensor.matmul(s_ps[:, j:j + 1], expg[j][:], ones_t[:],
                             start=True, stop=True)
        s_sb = wpool.tile([P, FI], f32, tag="ssb")
        nc.vector.tensor_copy(s_sb[:], s_ps[:])
        rs = wpool.tile([P, FI], f32, tag="rs")
        nc.vector.reciprocal(rs[:], s_sb[:])

        # ================= PASS 2: q, a, end-proj, layernorm =================
        for ch in range(NCH):
            qin, sin, xts = [], [], []
            for i in range(FI):
                t = cpool.tile([P, CHUNK], bf16, tag=f"cin{i}")
                nc.sync.dma_start(t[:], cT_own[i * P:(i + 1) * P,
                                               ch * CHUNK:(ch + 1) * CHUNK])
                qin.append(t)
                t = spool.tile([P, CHUNK], bf16, tag=f"s{i}")
                nc.sync.dma_start(t[:], sT[i * P:(i + 1) * P,
                                           ch * CHUNK:(ch + 1) * CHUNK])
                sin.append(t)
            for tb in range(NTB):
                t = xpool.tile([P, C], f32, tag=f"x{tb}")
                blk = ch * NTB + tb
                nc.sync.dma_start(t[:], x[blk * P:(blk + 1) * P, :])
                xts.append(t)
            qts = []
            for fo in range(FI):
                pst = ps_proj.tile([P, CHUNK], f32, tag="pp")
                for fi in range(FI):
                    nc.tensor.matmul(pst[:], Wown_t[fi][:, fo * P:(fo + 1) * P],
                                     qin[fi][:], start=(fi == 0), stop=(fi == FI - 1))
                t = ctq.tile([P, CHUNK], bf16, tag=f"ct{fo}")
                nc.scalar.activation(t[:], pst[:], AF.Relu,
                                     bias=bown_t[:, fo:fo + 1])
                qts.append(t)
            ats = []
            for j in range(FI):
                pa = ps_work.tile([P, CHUNK], f32, tag="pw")
                nc.tensor.matmul(pa[:], expg[j][:], qts[j][:],
                                 start=True, stop=True)
                t = apool.tile([P, CHUNK], bf16, tag=f"a{j}")
                nc.vector.tensor_scalar_mul(t[:], pa[:], rs[:, j:j + 1])
                ats.append(t)
            for tb in range(NTB):
                blk = ch * NTB + tb
                po = ps_work.tile([P, C], f32, tag="pw")
                for fi in range(FI):
                    nc.tensor.matmul(po[:], sin[fi][:, tb * P:(tb + 1) * P],
                                     We_t[fi][:], start=(fi == 0), stop=False)
                for j in range(FI):
                    nc.tensor.matmul(po[:], ats[j][:, tb * P:(tb + 1) * P],
                                     We_t[FI + j][:], start=False,
                                     stop=(j == FI - 1))
                # layernorm along free axis; var = E[z^2] - mu^2
                t_sb = opool.tile([P, C], f32, tag="t")
                s1 = opool.tile([P, 1], f32, tag="s1")
                nc.vector.tensor_add(t_sb[:], po[:], xts[tb][:])
                nc.vector.reduce_sum(s1[:], t_sb[:],
                                     axis=mybir.AxisListType.X)
                sq = opool.tile([P, C], f32, tag="sq")
                s2 = opool.tile([P, 1], f32, tag="s2")
                nc.scalar.activation(sq[:], t_sb[:], AF.Square,
                                     accum_out=s2[:])
                mu = opool.tile([P, 1], f32, tag="mu")
                nc.vector.tensor_scalar_mul(mu[:], s1[:], 1.0 / C)
                e2 = opool.tile([P, 1], f32, tag="e2")
                nc.vector.tensor_scalar_mul(e2[:], s2[:], 1.0 / C)
                musq = opool.tile([P, 1], f32, tag="musq")
                nc.vector.tensor_mul(musq[:], mu[:], mu[:])
                var = opool.tile([P, 1], f32, tag="var")
                nc.vector.tensor_sub(var[:], e2[:], musq[:])
                std = opool.tile([P, 1], f32, tag="std")
                nc.scalar.activation(std[:], var[:], AF.Sqrt, bias=eps_t[:])
                rstd = opool.tile([P, 1], f32, tag="rstd")
                nc.vector.reciprocal(rstd[:], std[:])
                o1 = opool.tile([P, C], f32, tag="o1")
                nc.vector.tensor_scalar(o1[:], t_sb[:], mu[:], rstd[:],
                                        op0=OP.subtract, op1=OP.mult)
                o2 = opool.tile([P, C], f32, tag="o2")
                nc.vector.tensor_mul(o2[:], o1[:], g_t[:])
                o3 = opool.tile([P, C], f32, tag="o3")
                nc.vector.tensor_add(o3[:], o2[:], bt_t[:])
                nc.sync.dma_start(out[blk * P:(blk + 1) * P, :], o3[:])

    nc.compile()
    return nc


def _get_nc():
    if "nc" not in _CACHE:
        _CACHE["nc"] = _build_nc()
    return _CACHE["nc"]


def _make_in_maps(inputs):
    """Host-side sharding: core 2*b+p gets batch b, path p."""
    f = {k: np.asarray(v) for k, v in inputs.items()}
    per_path = [
        # (c_own, c_oth, W_own, b_own, W_oth, b_oth, Wkv, We, be, g, bt, s, x)
        (f["c1"], f["c2"], f["W3"], f["b3"], f["W4"], f["b4"], f["Wkv2"],
         f["We1"], f["be1"], f["g1"], f["bt1"], f["s1"], f["x1"]),
        (f["c2"], f["c1"], f["W4"], f["b4"], f["W3"], f["b3"], f["Wkv1"],
         f["We2"], f["be2"], f["g2"], f["bt2"], f["s2"], f["x2"]),
    ]
    in_maps = []
    for b in range(B):
        for p in range(2):
            (c_own, c_oth, W_own, b_own_v, W_oth, b_oth_v, Wkv, We, be, g, bt,
             s, xx) = per_path[p]
            m = {
                "cT_own": np.ascontiguousarray(c_own[b].T).astype(BF16),
                "cT_oth": np.ascontiguousarray(c_oth[b].T).astype(BF16),
                "sT": np.ascontiguousarray(s[b].T).astype(BF16),
                "x": np.ascontiguousarray(xx[b] + be[None, :]).astype(np.float32),
                "W_own": W_own.astype(BF16),
                "W_oth": W_oth.astype(BF16),
                "b_own": np.ascontiguousarray(
                    b_own_v.reshape(FI, P).T).astype(np.float32),
                "b_oth": np.ascontiguousarray(
                    b_oth_v.reshape(FI, P).T).astype(np.float32),
                "Wkv": Wkv.astype(BF16),
                "We": We.astype(BF16),
                "g_rep": np.ascontiguousarray(
                    np.broadcast_to(g, (P, C))).astype(np.float32),
                "bt_rep": np.ascontiguousarray(
                    np.broadcast_to(bt, (P, C))).astype(np.float32),
            }
            in_maps.append(m)
    return in_maps


def _ensure_ntff_hook():
    """Register the axon NTFF profile hook if the image's antenv lacks it."""
    import sys
    import types
    try:
        from antenv.axon_hooks import get_axon_ntff_profile_hook  # noqa: F401
        return
    except ImportError:
        pass
    try:
        from trn_agent_boot.trn_boot import _ntff_profile_via_ctypes
        hook = _ntff_profile_via_ctypes("/opt/axon/libaxon_pjrt.so")
    except Exception:
        hook = None
    holder = {"h": hook}
    mod = types.ModuleType("antenv.axon_hooks")
    mod.set_axon_ntff_profile_hook = lambda h: holder.__setitem__("h", h)
    mod.get_axon_ntff_profile_hook = lambda: holder.get("h")
    try:
        import antenv
        antenv.axon_hooks = mod
    except ImportError:
        pkg = types.ModuleType("antenv")
        pkg.axon_hooks = mod
        sys.modules["antenv"] = pkg
    sys.modules["antenv.axon_hooks"] = mod


def kernel(**inputs):
    global LAST_RESULTS
    from concourse.bass_utils import run_bass_kernel_spmd

    nc = _get_nc()
    in_maps = _make_in_maps(inputs)
    trace = bool(int(os.environ.get("BASS_KERNEL_TRACE", "0")))
    if trace:
        _ensure_ntff_hook()
    res = run_bass_kernel_spmd(nc, in_maps, list(range(NCORES)), trace=trace)
    LAST_RESULTS = res
    out1 = np.stack([res.results[2 * b]["out"] for b in range(B)])
    out2 = np.stack([res.results[2 * b + 1]["out"] for b in range(B)])
    return out1.astype(np.float32), out2.astype(np.float32)
